# revision 1
# baseline (speedup 1.0000x reference)
"""SSA loss kernel for Trainium2 (8 NeuronCores, SPMD via run_bass_kernel_spmd).

Math: for L2-normalized rows a_b = x_m[b]/||x_m[b]||, c_b = x_n[b]/||x_n[b]||:
    sum((C_m - C_n)^2) = sum_b (||a_b||^4 + ||c_b||^4 - 2 (a_b . c_b)^2)
                       = 2*B - 2 * sum_b (x_m[b].x_n[b])^2 / (||x_m[b]||^2 ||x_n[b]||^2)
so only per-row squared norms and pairwise dots are needed — the [B,D,D]
correlation tensors are never materialized, and the total compute is ~0.4
MFLOP on 768 KB of input.

Distribution strategy (KERNEL_MODE, default "rep7"): at this problem size
the kernel is completely dominated by fixed per-launch costs, and a
measurement on this stack showed a 12-byte AllGather costs ~40 us of
collective-engine time — several times the entire remaining kernel. So the
chosen strategy is replication: every core receives the full (bf16-packed)
inputs and computes the full [3] output redundantly with zero cross-core
traffic; the host reads core 0's output. The batch-sharded data-parallel
variant from the sharding hint (16 rows/core + AllGather of the 3 pairwise
partial sums + on-device sqrt/focal/combine) is kept as build("shard"); it
measures ~82-116 us vs ~19 us for the replicated path, entirely due to
the collective floor.

Device program (one table-set, no cross-engine accumulator conflicts;
input DMAs fanned across the sync and gpsimd DGE issuers):
  - ScalarE: dummy Sqrt (pins the sqrt+square ACT table set, single load),
    3x Square with accum_out for the per-row squared norms
  - DVE: 3 bf16 cross-products, one fold-add halving the reduce length,
    then one grouped reduce for the pairwise dots; reciprocal + fused
    scalar_tensor_tensor ops for per-row ghat^2; final F*Csym combine as
    one fused scalar_tensor_tensor with accum_out; the
    focal weights ln(1+0.5*relu(l_m-l_n)) as a deg-4 Horner polynomial
    (exact 0 for l_m <= l_n, matching the reference's thresholded branch)
  - PE: tiny matmuls for the loss-difference outer product, the batch
    reduction of ghat^2, and the pair->modality-pair expansion
  - ScalarE: c = Sqrt(2B - 2T) fused into one activation (scale/bias)

Notes from bring-up on this runtime: InstTensorTensorReduce and GpSimd
tensor ops crash the exec unit (avoid); scalar_tensor_tensor with
accum_out works on DVE but DVE/ACT accum_out users serialize against each
other, so only ScalarE uses accum_out concurrently here.
"""

import numpy as np

import concourse.bass as bass
import concourse.bacc as bacc
import concourse.tile as tile
import concourse.mybir as mybir
from concourse import bass_utils

F32 = mybir.dt.float32
ALU = mybir.AluOpType
ACT = mybir.ActivationFunctionType

B, D = 128, 512
NCORES = 8
ROWS = B // NCORES          # 16 rows per core
CHUNK = 8                   # split each row of 512 into 8 chunks of 64
FREE = D // CHUNK           # 64
P = ROWS * CHUNK            # 128 partitions

# aux column layout
IND_C = 0                   # [:, 0:16]  indicator: IND[p, b] = (p // 8 == b)
ONES_C = 16                 # [:, 16]    ones column
M_C = 17                    # [0:3, 17:20] pair->modality incidence
ONES3_C = 20                # [0:1, 20:23] ones row (K=1 matmul operand)
L13_C = 23                  # [0:1, 23:26] losses in one partition
B2_C = 26                   # [:, 26]    constant 2B (Ln bias)
R0_C = 27                   # [:, 27]    rsqrt Newton seed 1/16
AUX_W = 28

THRESHOLD = 1e-12

_CACHE = {}


def build(mode="shard"):
    """Build and compile the 8-core Bass program (data-independent).

    mode="shard": data-parallel over batch + AllGather of partial sums.
    mode="rep":   every core computes the full loss redundantly (no
                  collective; the 12-byte AllGather costs ~40us on this
                  runtime, far more than the whole replicated compute).
    """
    if mode == "rep":
        return build_rep()
    if mode == "rep2":
        return build_rep2()
    if mode == "rep3":
        return build_rep3()
    if mode == "rep4":
        return build_rep4()
    if mode == "rep5":
        return build_rep4(multi_dma=True)
    if mode == "rep6":
        return build_rep4(multi_dma=True, fp8=True)
    if mode == "rep7":
        return build_rep4(multi_dma=True, fold=True, prodfuse=True)
    if mode == "rep8":
        return build_rep4(multi_dma=True, fold=True, prodfuse=True, outswdge=True)
    if mode == "rep9":
        return build_rep4(multi_dma=True, fold=True, prodfuse=True, v9=True)
    if mode == "rep10":
        return build_rep4(multi_dma=True, fold=True, prodfuse=True, v9=True, gmul=True)
    nc = bacc.Bacc(
        "TRN2",
        target_bir_lowering=False,
        debug=False,
        enable_asserts=False,
        num_devices=NCORES,
    )
    xcat_d = nc.dram_tensor("xcat", [P, 3 * FREE], F32, kind="ExternalInput").ap()
    aux_d = nc.dram_tensor("aux", [P, AUX_W], F32, kind="ExternalInput").ap()
    out_d = nc.dram_tensor("ssa", [3, 1], F32, kind="ExternalOutput").ap()

    with tile.TileContext(nc) as tc:
        with (
            tc.tile_pool(name="io", bufs=1) as iop,
            tc.tile_pool(name="work", bufs=1) as wp,
            tc.tile_pool(name="psum", bufs=1, space="PSUM") as pp,
            tc.tile_pool(name="dram", bufs=1, space="DRAM") as dp,
        ):
            xc = iop.tile([P, 3 * FREE], F32)
            aux = iop.tile([P, AUX_W], F32)
            nc.sync.dma_start(xc[:], xcat_d[:])
            nc.sync.dma_start(aux[:], aux_d[:])

            ind = aux[:, IND_C : IND_C + ROWS]
            ones_col = aux[:, ONES_C : ONES_C + 1]
            m_inc = aux[0:3, M_C : M_C + 3]
            ones3 = aux[0:1, ONES3_C : ONES3_C + 3]
            l13 = aux[0:1, L13_C : L13_C + 3]
            b2 = aux[:, B2_C : B2_C + 1]

            # ---- focal weights F[m,n] = (d>thr) * ln(0.5*max(d,thr)+1),
            # d = l_m - l_n. Depends only on aux -> scheduled early, so the
            # ACT table load overlaps the feature DMA/stats work.
            negl13 = wp.tile([1, 3], F32)
            nc.vector.tensor_scalar_mul(negl13[:], l13, -1.0)
            psD = pp.tile([3, 3], F32)
            nc.tensor.matmul(psD[:], l13, ones3, start=True, stop=False)
            nc.tensor.matmul(psD[:], ones3, negl13[:], start=False, stop=True)
            dmax = wp.tile([3, 3], F32)
            nc.vector.tensor_scalar_max(dmax[:], psD[:], THRESHOLD)
            lnf = wp.tile([3, 3], F32)
            nc.scalar.activation(lnf[:], dmax[:], ACT.Ln, bias=1.0, scale=0.5)
            mask = wp.tile([3, 3], F32)
            nc.vector.tensor_scalar(mask[:], psD[:], THRESHOLD, None, ALU.is_gt)
            focal = wp.tile([3, 3], F32)
            nc.vector.tensor_mul(focal[:], lnf[:], mask[:])

            # ---- per-(row,chunk) stats: 3 squared-norm partials + 3 dot partials
            scratch = wp.tile([P, FREE], F32)
            stats = wp.tile([P, 6], F32)
            mods = [xc[:, i * FREE : (i + 1) * FREE] for i in range(3)]
            pairs = [(0, 0), (1, 1), (2, 2), (0, 1), (0, 2), (1, 2)]
            for k, (i, j) in enumerate(pairs):
                nc.vector.tensor_mul(scratch[:], mods[i], mods[j])
                nc.vector.tensor_reduce(
                    stats[:, k : k + 1], scratch[:],
                    axis=mybir.AxisListType.X, op=ALU.add,
                )

            # ---- group-sum chunks: rowstats[b, k] = sum_c stats[8b+c, k]
            ps1 = pp.tile([ROWS, 6], F32)
            nc.tensor.matmul(ps1[:], ind, stats[:], start=True, stop=True)

            # ---- per-row ghat^2 terms: q^2 * (1/n2_i) * (1/n2_j)
            rs = wp.tile([ROWS, 6], F32)
            nc.vector.tensor_copy(rs[:], ps1[:])
            recips = wp.tile([ROWS, 3], F32)
            nc.vector.reciprocal(recips[:], rs[:, 0:3])
            q2 = wp.tile([ROWS, 3], F32)
            nc.vector.tensor_mul(q2[:], rs[:, 3:6], rs[:, 3:6])
            tmp3 = wp.tile([ROWS, 3], F32)
            terms = wp.tile([ROWS, 3], F32)
            for k, (i, j) in enumerate([(0, 1), (0, 2), (1, 2)]):
                nc.vector.tensor_mul(
                    tmp3[:, k : k + 1], q2[:, k : k + 1], recips[:, i : i + 1]
                )
                nc.vector.tensor_mul(
                    terms[:, k : k + 1], tmp3[:, k : k + 1], recips[:, j : j + 1]
                )

            # ---- local partial T_k = sum over this core's rows
            ps2 = pp.tile([3, 1], F32)
            nc.tensor.matmul(ps2[:], terms[:], ones_col[0:ROWS, :], start=True, stop=True)

            # ---- AllGather the 8x[3] partials (DRAM bounce buffers)
            cc_in = dp.tile([3, 1], F32)
            cc_out = dp.tile([NCORES * 3, 1], F32)
            t31 = wp.tile([3, 1], F32)
            nc.vector.tensor_copy(t31[:], ps2[:])
            nc.sync.dma_start(cc_in[:], t31[:])
            nc.gpsimd.collective_compute(
                "AllGather",
                ALU.bypass,
                replica_groups=[list(range(NCORES))],
                ins=[cc_in.opt()],
                outs=[cc_out.opt()],
            )
            t83 = wp.tile([NCORES, 3], F32)
            nc.sync.dma_start(t83[:], cc_out[:])

            # ---- total T, then c_k = sqrt(2B - 2*T_k) = exp(0.5*ln(2B - 2*T_k))
            psT = pp.tile([3, 1], F32)
            nc.tensor.matmul(psT[:], t83[:], ones_col[0:NCORES, :], start=True, stop=True)
            lns = wp.tile([3, 1], F32)
            nc.scalar.activation(lns[:], psT[:], ACT.Ln, bias=b2[0:3, :], scale=-2.0)
            csq = wp.tile([3, 1], F32)
            nc.scalar.activation(csq[:], lns[:], ACT.Exp, bias=0.0, scale=0.5)

            # ---- Csym[m,n] = sum_k M[k,m] M[k,n] c_k  (off-diag = c of that pair)
            cb = wp.tile([3, 3], F32)
            nc.vector.tensor_scalar_mul(cb[:], m_inc, csq[:])
            psC = pp.tile([3, 3], F32)
            nc.tensor.matmul(psC[:], m_inc, cb[:], start=True, stop=True)

            # ---- ssa[m] = sum_n F[m,n] * Csym[m,n]
            prod = wp.tile([3, 3], F32)
            nc.vector.tensor_mul(prod[:], focal[:], psC[:])
            ssa31 = wp.tile([3, 1], F32)
            nc.vector.tensor_reduce(ssa31[:], prod[:], axis=mybir.AxisListType.X, op=ALU.add)
            nc.sync.dma_start(out_d[:], ssa31[:])

    nc.compile()
    return nc


def build_rep():
    nc = bacc.Bacc(
        "TRN2",
        target_bir_lowering=False,
        debug=False,
        enable_asserts=False,
        num_devices=NCORES,
    )
    xcat_d = nc.dram_tensor("xcat", [B, 3 * D], F32, kind="ExternalInput").ap()
    aux_d = nc.dram_tensor("aux", [P, AUX_W], F32, kind="ExternalInput").ap()
    out_d = nc.dram_tensor("ssa", [3, 1], F32, kind="ExternalOutput").ap()

    with tile.TileContext(nc) as tc:
        with (
            tc.tile_pool(name="io", bufs=1) as iop,
            tc.tile_pool(name="work", bufs=1) as wp,
            tc.tile_pool(name="psum", bufs=1, space="PSUM") as pp,
        ):
            xc = iop.tile([B, 3 * D], F32)
            aux = iop.tile([P, AUX_W], F32)
            # split the 768KB load into per-modality-half DMAs so they fan
            # out across DMA queues and overlap with the stats compute
            nhalf = D // 2
            for i in range(3):
                for h in range(2):
                    lo = i * D + h * nhalf
                    nc.sync.dma_start(xc[:, lo : lo + nhalf], xcat_d[:, lo : lo + nhalf])
            nc.sync.dma_start(aux[:], aux_d[:])

            ones_col = aux[:, ONES_C : ONES_C + 1]
            m_inc = aux[0:3, M_C : M_C + 3]
            ones3 = aux[0:1, ONES3_C : ONES3_C + 3]
            l13 = aux[0:1, L13_C : L13_C + 3]
            b2 = aux[:, B2_C : B2_C + 1]

            # focal weights (identical to shard mode)
            negl13 = wp.tile([1, 3], F32)
            nc.vector.tensor_scalar_mul(negl13[:], l13, -1.0)
            psD = pp.tile([3, 3], F32)
            nc.tensor.matmul(psD[:], l13, ones3, start=True, stop=False)
            nc.tensor.matmul(psD[:], ones3, negl13[:], start=False, stop=True)
            dmax = wp.tile([3, 3], F32)
            nc.vector.tensor_scalar_max(dmax[:], psD[:], THRESHOLD)
            lnf = wp.tile([3, 3], F32)
            nc.scalar.activation(lnf[:], dmax[:], ACT.Ln, bias=1.0, scale=0.5)
            mask = wp.tile([3, 3], F32)
            nc.vector.tensor_scalar(mask[:], psD[:], THRESHOLD, None, ALU.is_gt)
            focal = wp.tile([3, 3], F32)
            nc.vector.tensor_mul(focal[:], lnf[:], mask[:])

            # per-row stats over the full batch: [128 rows, 512] per modality,
            # halves reduced separately (pipelines with the half DMAs), then added
            scratch = wp.tile([B, nhalf], F32)
            statsh = wp.tile([B, 12], F32)
            stats = wp.tile([B, 6], F32)
            mods = [
                (xc[:, i * D : i * D + nhalf], xc[:, i * D + nhalf : (i + 1) * D])
                for i in range(3)
            ]
            pairs = [(0, 0), (1, 1), (2, 2), (0, 1), (0, 2), (1, 2)]
            for k, (i, j) in enumerate(pairs):
                for h in range(2):
                    nc.vector.tensor_mul(scratch[:], mods[i][h], mods[j][h])
                    nc.vector.tensor_reduce(
                        statsh[:, 6 * h + k : 6 * h + k + 1], scratch[:],
                        axis=mybir.AxisListType.X, op=ALU.add,
                    )
            nc.vector.tensor_add(stats[:], statsh[:, 0:6], statsh[:, 6:12])

            recips = wp.tile([B, 3], F32)
            nc.vector.reciprocal(recips[:], stats[:, 0:3])
            q2 = wp.tile([B, 3], F32)
            nc.vector.tensor_mul(q2[:], stats[:, 3:6], stats[:, 3:6])
            tmp3 = wp.tile([B, 3], F32)
            terms = wp.tile([B, 3], F32)
            for k, (i, j) in enumerate([(0, 1), (0, 2), (1, 2)]):
                nc.vector.tensor_mul(
                    tmp3[:, k : k + 1], q2[:, k : k + 1], recips[:, i : i + 1]
                )
                nc.vector.tensor_mul(
                    terms[:, k : k + 1], tmp3[:, k : k + 1], recips[:, j : j + 1]
                )

            # total T over the whole batch in one matmul (no collective)
            psT = pp.tile([3, 1], F32)
            nc.tensor.matmul(psT[:], terms[:], ones_col[0:B, :], start=True, stop=True)
            lns = wp.tile([3, 1], F32)
            nc.scalar.activation(lns[:], psT[:], ACT.Ln, bias=b2[0:3, :], scale=-2.0)
            csq = wp.tile([3, 1], F32)
            nc.scalar.activation(csq[:], lns[:], ACT.Exp, bias=0.0, scale=0.5)

            cb = wp.tile([3, 3], F32)
            nc.vector.tensor_scalar_mul(cb[:], m_inc, csq[:])
            psC = pp.tile([3, 3], F32)
            nc.tensor.matmul(psC[:], m_inc, cb[:], start=True, stop=True)
            prod = wp.tile([3, 3], F32)
            nc.vector.tensor_mul(prod[:], focal[:], psC[:])
            ssa31 = wp.tile([3, 1], F32)
            nc.vector.tensor_reduce(ssa31[:], prod[:], axis=mybir.AxisListType.X, op=ALU.add)
            nc.sync.dma_start(out_d[:], ssa31[:])

    nc.compile()
    return nc


def build_rep2():
    """Replicated, bf16 feature path, no collective, no tail ACT ops.

    - squared norms on ScalarE via activation(Square, accum_out)
    - cross dots on DVE in bf16 (2x rate), grouped reduces
    - focal ln(1+0.5*relu(d)) as a DVE atanh polynomial (exact 0 at d<=0)
    - c = sqrt(S) via DVE Newton rsqrt (seed 1/16; S = 2B - 2T is always
      near 256 for L2-normalized rows)
    pair order here: k0=(A,V), k1=(V,T), k2=(A,T)
    """
    BF16 = mybir.dt.float8e4 if fp8 else mybir.dt.bfloat16
    COEFFS = LN_COEFFS4  # deg-4: best op-count/precision tradeoff
    nc = bacc.Bacc(
        "TRN2",
        target_bir_lowering=False,
        debug=False,
        enable_asserts=False,
        num_devices=NCORES,
    )
    xcat_d = nc.dram_tensor("xcat", [B, 3 * D], BF16, kind="ExternalInput").ap()
    aux_d = nc.dram_tensor("aux", [P, AUX_W], F32, kind="ExternalInput").ap()
    out_d = nc.dram_tensor("ssa", [3, 1], F32, kind="ExternalOutput").ap()

    with tile.TileContext(nc) as tc:
        with (
            tc.tile_pool(name="io", bufs=1) as iop,
            tc.tile_pool(name="work", bufs=1) as wp,
            tc.tile_pool(name="psum", bufs=1, space="PSUM") as pp,
        ):
            xc = iop.tile([B, 3 * D], BF16)
            aux = iop.tile([P, AUX_W], F32)
            nc.sync.dma_start(aux[:], aux_d[:])
            for i in range(3):
                nc.sync.dma_start(
                    xc[:, i * D : (i + 1) * D], xcat_d[:, i * D : (i + 1) * D]
                )

            ones_col = aux[:, ONES_C : ONES_C + 1]
            m_inc = aux[0:3, M_C : M_C + 3]
            ones3 = aux[0:1, ONES3_C : ONES3_C + 3]
            l13 = aux[0:1, L13_C : L13_C + 3]
            r0 = aux[0:3, R0_C : R0_C + 1]
            A = xc[:, 0:D]
            V = xc[:, D : 2 * D]
            T = xc[:, 2 * D : 3 * D]

            # ---- loss-difference matrix D[m,n] = l_m - l_n (PE outer products)
            negl13 = wp.tile([1, 3], F32)
            nc.vector.tensor_scalar_mul(negl13[:], l13, -1.0)
            psD = pp.tile([3, 3], F32)
            nc.tensor.matmul(psD[:], l13, ones3, start=True, stop=False)
            nc.tensor.matmul(psD[:], ones3, negl13[:], start=False, stop=True)

            # ---- focal = ln(1 + y), y = 0.5*relu(D), via 2*atanh(u), u=y/(2+y)
            y = wp.tile([3, 3], F32)
            nc.vector.tensor_scalar(y[:], psD[:], 0.0, 0.5, ALU.max, ALU.mult)
            t2 = wp.tile([3, 3], F32)
            nc.vector.tensor_scalar_add(t2[:], y[:], 2.0)
            rt = wp.tile([3, 3], F32)
            nc.vector.reciprocal(rt[:], t2[:])
            u = wp.tile([3, 3], F32)
            nc.vector.tensor_mul(u[:], y[:], rt[:])
            u2 = wp.tile([3, 3], F32)
            nc.vector.tensor_mul(u2[:], u[:], u[:])
            pl = wp.tile([3, 3], F32)
            nc.vector.tensor_scalar(pl[:], u2[:], 1.0 / 9.0, 1.0 / 7.0, ALU.mult, ALU.add)
            for cst in (0.2, 1.0 / 3.0, 1.0):
                nc.vector.tensor_mul(pl[:], pl[:], u2[:])
                nc.vector.tensor_scalar_add(pl[:], pl[:], cst)
            uu = wp.tile([3, 3], F32)
            nc.vector.tensor_add(uu[:], u[:], u[:])
            focal = wp.tile([3, 3], F32)
            nc.vector.tensor_mul(focal[:], uu[:], pl[:])

            # ---- squared norms on ScalarE (Square + row-accumulate)
            sqs = wp.tile([B, 3 * D], BF16)
            n2t = wp.tile([B, 3], F32)
            for i, mod in enumerate((A, V, T)):
                nc.scalar.activation(
                    sqs[:, i * D : (i + 1) * D], mod,
                    ACT.Square, accum_out=n2t[:, i : i + 1],
                )

            # ---- cross dots on DVE: (A,V)*(V,T) grouped, then A*T
            scr = wp.tile([B, 2 * D], BF16)
            qt = wp.tile([B, 3], F32)
            nc.vector.tensor_mul(scr[:], xc[:, 0 : 2 * D], xc[:, D : 3 * D])
            nc.vector.tensor_reduce(
                qt[:, 0:2], scr[:].rearrange("p (g d) -> p g d", d=D),
                axis=mybir.AxisListType.X, op=ALU.add,
            )
            scr2 = wp.tile([B, D], BF16)
            nc.vector.tensor_mul(scr2[:], A, T)
            nc.vector.tensor_reduce(
                qt[:, 2:3], scr2[:], axis=mybir.AxisListType.X, op=ALU.add,
            )

            # ---- per-row ghat^2: pairs k0=(A,V) k1=(V,T) k2=(A,T)
            recips = wp.tile([B, 3], F32)
            nc.vector.reciprocal(recips[:], n2t[:])
            q2 = wp.tile([B, 3], F32)
            nc.vector.tensor_mul(q2[:], qt[:], qt[:])
            tmp3 = wp.tile([B, 3], F32)
            terms = wp.tile([B, 3], F32)
            for k, (i, j) in enumerate([(0, 1), (1, 2), (0, 2)]):
                nc.vector.tensor_mul(
                    tmp3[:, k : k + 1], q2[:, k : k + 1], recips[:, i : i + 1]
                )
                nc.vector.tensor_mul(
                    terms[:, k : k + 1], tmp3[:, k : k + 1], recips[:, j : j + 1]
                )

            # ---- T totals over the batch, S = 2B - 2T
            psT = pp.tile([3, 1], F32)
            nc.tensor.matmul(psT[:], terms[:], ones_col[0:B, :], start=True, stop=True)
            s = wp.tile([3, 1], F32)
            nc.vector.tensor_scalar(s[:], psT[:], -2.0, float(2 * B), ALU.mult, ALU.add)

            # ---- c = sqrt(s) = s * rsqrt(s), Newton from r0 = 1/16
            r = r0
            for it in range(3):
                rr = wp.tile([3, 1], F32, name=f"rr{it}")
                nc.vector.tensor_mul(rr[:], r, r)
                rs = wp.tile([3, 1], F32, name=f"rs{it}")
                nc.vector.tensor_mul(rs[:], rr[:], s[:])
                w = wp.tile([3, 1], F32, name=f"w{it}")
                nc.vector.tensor_scalar(w[:], rs[:], -0.5, 1.5, ALU.mult, ALU.add)
                rn = wp.tile([3, 1], F32, name=f"rn{it}")
                nc.vector.tensor_mul(rn[:], r, w[:])
                r = rn[:]
            csq = wp.tile([3, 1], F32)
            nc.vector.tensor_mul(csq[:], s[:], r)

            # ---- Csym and final combine
            cb = wp.tile([3, 3], F32)
            nc.vector.tensor_scalar_mul(cb[:], m_inc, csq[:])
            psC = pp.tile([3, 3], F32)
            nc.tensor.matmul(psC[:], m_inc, cb[:], start=True, stop=True)
            prod = wp.tile([3, 3], F32)
            nc.vector.tensor_mul(prod[:], focal[:], psC[:])
            ssa31 = wp.tile([3, 1], F32)
            nc.vector.tensor_reduce(ssa31[:], prod[:], axis=mybir.AxisListType.X, op=ALU.add)
            nc.sync.dma_start(out_d[:], ssa31[:])

    nc.compile()
    return nc


LN_COEFFS4 = [  # deg-4 variant, max abs err 1.6e-6 on [0, 0.5]
    0.08226592154386331, -0.20860710798494606, 0.3267089860977848,
    -0.49961183645537066, 0.9999962807184205,
]

LN_COEFFS = [  # ln(1+y) = y*q(y), y in [0,0.5]; Horner s=(s+c)*y steps
    0.038723589983158405, -0.11377238656354291, 0.1845474004267164,
    -0.24753098551625669, 0.3331394148916809, -0.49999418623410385,
    0.9999999712565049,
]


def build_rep3():
    """Replicated bf16 path v3: fused DVE dots via scalar_tensor_tensor
    accum, ACT squares + single-table Sqrt, DVE Horner focal."""
    BF16 = mybir.dt.float8e4 if fp8 else mybir.dt.bfloat16
    COEFFS = LN_COEFFS4  # deg-4: best op-count/precision tradeoff
    nc = bacc.Bacc(
        "TRN2",
        target_bir_lowering=False,
        debug=False,
        enable_asserts=False,
        num_devices=NCORES,
    )
    xcat_d = nc.dram_tensor("xcat", [B, 3 * D], BF16, kind="ExternalInput").ap()
    aux_d = nc.dram_tensor("aux", [P, AUX_W], F32, kind="ExternalInput").ap()
    out_d = nc.dram_tensor("ssa", [3, 1], F32, kind="ExternalOutput").ap()

    with tile.TileContext(nc) as tc:
        with (
            tc.tile_pool(name="io", bufs=1) as iop,
            tc.tile_pool(name="work", bufs=1) as wp,
            tc.tile_pool(name="psum", bufs=1, space="PSUM") as pp,
        ):
            xc = iop.tile([B, 3 * D], BF16)
            aux = iop.tile([P, AUX_W], F32)
            for i in range(3):
                nc.sync.dma_start(
                    xc[:, i * D : (i + 1) * D], xcat_d[:, i * D : (i + 1) * D]
                )
            nc.sync.dma_start(aux[:], aux_d[:])

            ones_col = aux[:, ONES_C : ONES_C + 1]
            m_inc = aux[0:3, M_C : M_C + 3]
            ones3 = aux[0:1, ONES3_C : ONES3_C + 3]
            l13 = aux[0:1, L13_C : L13_C + 3]
            A = xc[:, 0:D]
            V = xc[:, D : 2 * D]
            T = xc[:, 2 * D : 3 * D]

            # squared norms on ScalarE (Square + row-accumulate)
            sqs = wp.tile([B, 3 * D], BF16)
            n2t = wp.tile([B, 3], F32)
            for i, mod in enumerate((A, V, T)):
                nc.scalar.activation(
                    sqs[:, i * D : (i + 1) * D], mod,
                    ACT.Square, accum_out=n2t[:, i : i + 1],
                )

            # cross dots, one fused DVE op each: out=(x*1)*y, accum=row-sum
            scr = wp.tile([B, 3 * D], BF16)
            qt = wp.tile([B, 3], F32)
            for k, (u, v) in enumerate(((A, V), (V, T), (A, T))):
                nc.vector.scalar_tensor_tensor(
                    scr[:, k * D : (k + 1) * D], u, 1.0, v,
                    op0=ALU.mult, op1=ALU.mult, accum_out=qt[:, k : k + 1],
                )

            # per-row ghat^2 terms: one fused op per pair
            recips = wp.tile([B, 3], F32)
            nc.vector.reciprocal(recips[:], n2t[:])
            q2 = wp.tile([B, 3], F32)
            nc.vector.tensor_mul(q2[:], qt[:], qt[:])
            terms = wp.tile([B, 3], F32)
            for k, (i, j) in enumerate([(0, 1), (1, 2), (0, 2)]):
                nc.vector.scalar_tensor_tensor(
                    terms[:, k : k + 1], q2[:, k : k + 1],
                    recips[:, i : i + 1], recips[:, j : j + 1],
                    op0=ALU.mult, op1=ALU.mult,
                )

            # batch totals, S = 2B - 2T, c = sqrt(S)
            psT = pp.tile([3, 1], F32)
            nc.tensor.matmul(psT[:], terms[:], ones_col[0:B, :], start=True, stop=True)
            s = wp.tile([3, 1], F32)
            nc.vector.tensor_scalar(s[:], psT[:], -2.0, float(2 * B), ALU.mult, ALU.add)
            csq = wp.tile([3, 1], F32)
            nc.scalar.activation(csq[:], s[:], ACT.Sqrt)

            # focal weights: D[m,n]=l_m-l_n via PE, ln(1+0.5*relu(D)) via
            # DVE Horner chain (exact 0 at D<=0); emitted late so the big
            # DVE ops above win scheduler priority
            negl13 = wp.tile([1, 3], F32)
            nc.vector.tensor_scalar_mul(negl13[:], l13, -1.0)
            psD = pp.tile([3, 3], F32)
            nc.tensor.matmul(psD[:], l13, ones3, start=True, stop=False)
            nc.tensor.matmul(psD[:], ones3, negl13[:], start=False, stop=True)
            y = wp.tile([3, 3], F32)
            nc.vector.tensor_scalar(y[:], psD[:], 0.0, 0.5, ALU.max, ALU.mult)
            focal = wp.tile([3, 3], F32)
            nc.vector.tensor_scalar_mul(focal[:], y[:], LN_COEFFS[0])
            for c in LN_COEFFS[1:]:
                nc.vector.scalar_tensor_tensor(
                    focal[:], focal[:], c, y[:], op0=ALU.add, op1=ALU.mult
                )

            # Csym and final combine
            cb = wp.tile([3, 3], F32)
            nc.vector.tensor_scalar_mul(cb[:], m_inc, csq[:])
            psC = pp.tile([3, 3], F32)
            nc.tensor.matmul(psC[:], m_inc, cb[:], start=True, stop=True)
            prod = wp.tile([3, 3], F32)
            nc.vector.tensor_mul(prod[:], focal[:], psC[:])
            ssa31 = wp.tile([3, 1], F32)
            nc.vector.tensor_reduce(ssa31[:], prod[:], axis=mybir.AxisListType.X, op=ALU.add)
            nc.sync.dma_start(out_d[:], ssa31[:])

    nc.compile()
    return nc


def build_rep4(multi_dma=False, fp8=False, fold=False, prodfuse=False, outswdge=False, v9=False, gmul=False):
    """rep3 + single ACT table set (dummy Sqrt first) and no DVE accum
    (plain muls + one grouped reduce) to avoid ACT/DVE accumulator
    serialization."""
    BF16 = mybir.dt.float8e4 if fp8 else mybir.dt.bfloat16
    COEFFS = LN_COEFFS4  # deg-4: best op-count/precision tradeoff
    nc = bacc.Bacc(
        "TRN2",
        target_bir_lowering=False,
        debug=False,
        enable_asserts=False,
        num_devices=NCORES,
    )
    xcat_d = nc.dram_tensor("xcat", [B, 3 * D], BF16, kind="ExternalInput").ap()
    aux_d = nc.dram_tensor("aux", [P, AUX_W], F32, kind="ExternalInput").ap()
    out_d = nc.dram_tensor("ssa", [3, 1], F32, kind="ExternalOutput").ap()

    with tile.TileContext(nc) as tc:
        with (
            tc.tile_pool(name="io", bufs=1) as iop,
            tc.tile_pool(name="work", bufs=1) as wp,
            tc.tile_pool(name="psum", bufs=1, space="PSUM") as pp,
        ):
            xc = iop.tile([B, 3 * D], BF16)
            aux = iop.tile([P, AUX_W], F32)
            if multi_dma:
                # fan the input transfers across the two free DGE issuers
                nc.sync.dma_start(xc[:, 0:D], xcat_d[:, 0:D])
                nc.gpsimd.dma_start(xc[:, D : 2 * D], xcat_d[:, D : 2 * D])
                nc.sync.dma_start(xc[:, 2 * D : 3 * D], xcat_d[:, 2 * D : 3 * D])
                nc.gpsimd.dma_start(aux[:], aux_d[:])
            else:
                nc.sync.dma_start(xc[:, 0 : 2 * D], xcat_d[:, 0 : 2 * D])
                nc.sync.dma_start(xc[:, 2 * D : 3 * D], xcat_d[:, 2 * D : 3 * D])
                nc.sync.dma_start(aux[:], aux_d[:])

            ones_col = aux[:, ONES_C : ONES_C + 1]
            m_inc = aux[0:3, M_C : M_C + 3]
            ones3 = aux[0:1, ONES3_C : ONES3_C + 3]
            l13 = aux[0:1, L13_C : L13_C + 3]
            b2 = aux[:, B2_C : B2_C + 1]
            A = xc[:, 0:D]
            V = xc[:, D : 2 * D]
            T = xc[:, 2 * D : 3 * D]

            # dummy Sqrt on an always-ready const so walrus picks the one
            # table set containing both sqrt and square (single load)
            dummy = wp.tile([1, 1], F32)
            nc.scalar.activation(dummy[:], nc.const_aps.tensor(1.0, (1, 1)), ACT.Sqrt)

            # squared norms on ScalarE (Square + row-accumulate)
            sqs = wp.tile([B, 3 * D], BF16)
            n2t = wp.tile([B, 3], F32)
            for i, mod in enumerate((A, V, T)):
                nc.scalar.activation(
                    sqs[:, i * D : (i + 1) * D], mod,
                    ACT.Square, accum_out=n2t[:, i : i + 1],
                )

            # cross dots: 3 plain muls (start as DMAs land) + one grouped reduce
            scr = wp.tile([B, 3 * D], BF16)
            qt = wp.tile([B, 3], F32)
            if gmul:
                # (A,V)*(V,T) as one 1024-col op: T lands with V, and one
                # wide op beats two 512-col ops on fixed overhead
                nc.vector.tensor_mul(
                    scr[:, 0 : 2 * D], xc[:, 0 : 2 * D], xc[:, D : 3 * D]
                )
                nc.vector.tensor_mul(scr[:, 2 * D : 3 * D], A, T)
            else:
                for k, (u, v) in enumerate(((A, V), (V, T), (A, T))):
                    nc.vector.tensor_mul(scr[:, k * D : (k + 1) * D], u, v)
            if fold:
                # halve the (slow) reduce length with one cheap add first
                fold_t = wp.tile([B, 3 * (D // 2)], BF16)
                sv = scr[:].rearrange("p (g d) -> p g d", d=D)
                fv = fold_t[:].rearrange("p (g d) -> p g d", d=D // 2)
                nc.vector.tensor_add(fv, sv[:, :, 0 : D // 2], sv[:, :, D // 2 : D])
                if v9:
                    fold2_t = wp.tile([B, 3 * (D // 4)], BF16)
                    f2v = fold2_t[:].rearrange("p (g d) -> p g d", d=D // 4)
                    nc.vector.tensor_add(
                        f2v, fv[:, :, 0 : D // 4], fv[:, :, D // 4 : D // 2]
                    )
                    nc.vector.tensor_reduce(
                        qt[:], f2v, axis=mybir.AxisListType.X, op=ALU.add
                    )
                else:
                    nc.vector.tensor_reduce(
                        qt[:], fv, axis=mybir.AxisListType.X, op=ALU.add
                    )
            else:
                nc.vector.tensor_reduce(
                    qt[:], scr[:].rearrange("p (g d) -> p g d", d=D),
                    axis=mybir.AxisListType.X, op=ALU.add,
                )

            # per-row ghat^2 terms: one fused op per pair
            recips = wp.tile([B, 3], F32)
            nc.vector.reciprocal(recips[:], n2t[:])
            q2 = wp.tile([B, 3], F32)
            nc.vector.tensor_mul(q2[:], qt[:], qt[:])
            terms = wp.tile([B, 3], F32)
            for k, (i, j) in enumerate([(0, 1), (1, 2), (0, 2)]):
                nc.vector.scalar_tensor_tensor(
                    terms[:, k : k + 1], q2[:, k : k + 1],
                    recips[:, i : i + 1], recips[:, j : j + 1],
                    op0=ALU.mult, op1=ALU.mult,
                )

            # batch totals, S = 2B - 2T, c = sqrt(S)
            psT = pp.tile([3, 1], F32)
            nc.tensor.matmul(psT[:], terms[:], ones_col[0:B, :], start=True, stop=True)
            csq = wp.tile([3, 1], F32)
            nc.scalar.activation(
                csq[:], psT[:], ACT.Sqrt, bias=b2[0:3, :], scale=-2.0
            )

            # focal weights: D[m,n]=l_m-l_n via PE, ln(1+0.5*relu(D)) via
            # DVE Horner chain (exact 0 at D<=0); emitted late so the big
            # DVE ops above win scheduler priority
            negl13 = wp.tile([1, 3], F32)
            nc.vector.tensor_scalar_mul(negl13[:], l13, -1.0)
            psD = pp.tile([3, 3], F32)
            nc.tensor.matmul(psD[:], l13, ones3, start=True, stop=False)
            nc.tensor.matmul(psD[:], ones3, negl13[:], start=False, stop=True)
            y = wp.tile([3, 3], F32)
            nc.vector.tensor_scalar(y[:], psD[:], 0.0, 0.5, ALU.max, ALU.mult)
            focal = wp.tile([3, 3], F32)
            if v9:
                # first Horner term straight from psD: parallel with y, one
                # less serial hop in the polynomial chain
                nc.vector.tensor_scalar(
                    focal[:], psD[:], 0.0, 0.5 * COEFFS[0], ALU.max, ALU.mult
                )
            else:
                nc.vector.tensor_scalar_mul(focal[:], y[:], COEFFS[0])
            for c in COEFFS[1:]:
                nc.vector.scalar_tensor_tensor(
                    focal[:], focal[:], c, y[:], op0=ALU.add, op1=ALU.mult
                )

            # Csym and final combine
            cb = wp.tile([3, 3], F32)
            nc.vector.tensor_scalar_mul(cb[:], m_inc, csq[:])
            psC = pp.tile([3, 3], F32)
            nc.tensor.matmul(psC[:], m_inc, cb[:], start=True, stop=True)
            prod = wp.tile([3, 3], F32)
            ssa31 = wp.tile([3, 1], F32)
            if prodfuse:
                nc.vector.scalar_tensor_tensor(
                    prod[:], focal[:], 1.0, psC[:],
                    op0=ALU.mult, op1=ALU.mult, accum_out=ssa31[:],
                )
            else:
                nc.vector.tensor_mul(prod[:], focal[:], psC[:])
                nc.vector.tensor_reduce(
                    ssa31[:], prod[:], axis=mybir.AxisListType.X, op=ALU.add
                )
            if outswdge:
                nc.gpsimd.dma_start(out_d[:], ssa31[:])
            else:
                nc.sync.dma_start(out_d[:], ssa31[:])

    nc.compile()
    return nc


def make_in_maps(x_audio, x_video, x_text, loss_audio, loss_video, loss_text,
                 mode="shard"):
    xa = np.ascontiguousarray(np.asarray(x_audio, dtype=np.float32))
    xv = np.ascontiguousarray(np.asarray(x_video, dtype=np.float32))
    xt = np.ascontiguousarray(np.asarray(x_text, dtype=np.float32))

    aux = np.zeros((P, AUX_W), dtype=np.float32)
    aux[np.arange(P), np.arange(P) // CHUNK] = 1.0            # IND
    aux[:, ONES_C] = 1.0
    if mode in ("rep2", "rep3", "rep4", "rep5", "rep6", "rep7", "rep8", "rep9", "rep10"):
        # pair order k0=(A,V), k1=(V,T), k2=(A,T)
        aux[0:3, M_C : M_C + 3] = [[1, 1, 0], [0, 1, 1], [1, 0, 1]]
    else:
        aux[0:3, M_C : M_C + 3] = [[1, 1, 0], [1, 0, 1], [0, 1, 1]]
    aux[:, ONES3_C : ONES3_C + 3] = 1.0
    aux[:, B2_C] = float(2 * B)
    aux[:, R0_C] = 1.0 / 16.0
    aux[0, L13_C : L13_C + 3] = [
        np.float32(np.asarray(loss_audio)),
        np.float32(np.asarray(loss_video)),
        np.float32(np.asarray(loss_text)),
    ]

    if mode == "rep":
        xcat = np.ascontiguousarray(np.concatenate([xa, xv, xt], axis=1))
        return [{"xcat": xcat, "aux": aux} for _ in range(NCORES)]

    if mode in ("rep2", "rep3", "rep4", "rep5", "rep6", "rep7", "rep8", "rep9", "rep10"):
        import ml_dtypes

        feat_dt = ml_dtypes.float8_e4m3 if mode == "rep6" else ml_dtypes.bfloat16
        xcat = np.ascontiguousarray(
            np.concatenate([xa, xv, xt], axis=1).astype(feat_dt)
        )
        return [{"xcat": xcat, "aux": aux} for _ in range(NCORES)]

    in_maps = []
    for r in range(NCORES):
        sl = slice(r * ROWS, (r + 1) * ROWS)
        xcat = np.concatenate(
            [xa[sl].reshape(P, FREE), xv[sl].reshape(P, FREE), xt[sl].reshape(P, FREE)],
            axis=1,
        )
        in_maps.append({"xcat": np.ascontiguousarray(xcat), "aux": aux})
    return in_maps


KERNEL_MODE = "rep7"


def _launch(nc, in_maps):
    res = bass_utils.run_bass_kernel_spmd(nc, in_maps, core_ids=list(range(NCORES)))
    return [
        np.asarray(res.results[c]["ssa"], dtype=np.float32).reshape(3).copy()
        for c in range(NCORES)
    ]


def kernel(x_audio, x_video, x_text, loss_audio, loss_video, loss_text):
    if "nc" not in _CACHE:
        _CACHE["nc"] = build(KERNEL_MODE)
    nc = _CACHE["nc"]
    in_maps = make_in_maps(x_audio, x_video, x_text, loss_audio, loss_video,
                           loss_text, mode=KERNEL_MODE)

    # Every core computes the identical full output. The runtime very
    # occasionally corrupts a first execution (or crashes), so take the
    # majority result across the 8 redundant cores and retry the launch
    # unless it is a clean, finite super-majority.
    from collections import Counter

    out = None
    last_exc = None
    for _attempt in range(3):
        try:
            outs = _launch(nc, in_maps)
        except Exception as exc:  # device hiccup: reset the client and retry
            last_exc = exc
            try:
                import jax
                import jax.extend as _jex

                jax.clear_caches()
                _jex.backend.clear_backends()
            except Exception:
                pass
            continue
        votes = Counter(o.tobytes() for o in outs)
        best, n = votes.most_common(1)[0]
        out = np.frombuffer(best, dtype=np.float32).reshape(3).copy()
        if n >= (NCORES // 2 + 1) and np.all(np.isfinite(out)):
            return out
    if out is not None:
        return out
    raise last_exc

